# revision 1
# baseline (speedup 1.0000x reference)
"""Trainium2 Bass kernel for DeformConv2d (DCNv2, modulated), 8-core SPMD.

Problem: input [4,64,128,128], offset [4,144,128,128] (8 offset groups x 9
taps x (dy,dx)), mask [4,72,128,128], weight [64,64,3,3], bias [64];
stride 1, pad 1, dil 1, weight groups 1. Output [4,64,128,128].

Sharding: 8 cores = (batch b, row-half h); each core computes 64 output rows
of one batch image.

Per-core algorithm (all on device):
  * padded bf16 image (pad 8; zero border) in SBUF, plus a pair-word packed
    "gather source" GS per 16-row block: for every offset group g, GpSimd
    core g's 16 partitions hold the group's 8 channels twice -- partitions
    c'=0..7 with the block window at row offset 0, partitions 8..15 shifted
    down one row.  Each partition stores TWO copies of the window (elems
    [0:N) and [1:N+1)) so any horizontal pair (x0,x0+1) is one aligned
    32-bit word.  One ap_gather index per sample position then fetches all
    4 bilinear corners (top pair from partitions 0..7, bottom pair from
    8..15).
  * DVE computes positions, trunc-floors, fractional weights, word indices.
  * TensorE replicates the per-(g,k,pixel) weight quads across the 8
    channels of each group (0/1 matrices), ScalarE evacuates psum -> bf16.
  * DVE multiplies gathered pairs by weights; TensorE contracts the
    (group, half, channel) x tap dimensions with the conv weights (signs
    folded into the stationary operand) accumulating the output in PSUM.
"""

import os
import sys
import numpy as np

sys.path.insert(0, "/opt/trn_rl_repo")

# ---------------- constants (hardcoded for this problem) ----------------
B, C, H, W = 4, 64, 128, 128
G, K, Cg = 8, 9, 8
P = 8                     # zero padding on every side (offsets stay < 6)
WPAD = W + 2 * P          # 144
ROWS_CORE = 64            # output rows per core
PIMG_ROWS = ROWS_CORE + 18  # 82 padded rows resident per core
PIMG_TAIL = 146           # slack so the +1-row shifted copy stays in bounds
NPIMG = PIMG_ROWS * WPAD + PIMG_TAIL  # 11954 elements per channel
RB = 16                   # output rows per block
NBLK = ROWS_CORE // RB    # 4
BLKPIX = RB * W           # 2048
HALF = BLKPIX // 2        # 1024
NWC = PIMG_ROWS * WPAD // 2   # 5904 words per copy (whole core window)
NWC2 = 2 * NWC            # 11808 words per partition
NPIX = ROWS_CORE * W      # 8192
CLAMP_Y = (0.001, PIMG_ROWS - 1.002)  # fy <= 80, fy+1 <= 81
CLAMP_X = (0.001, WPAD - 1.002)

_BUILT = {}


def _build_nc(round_cast=True):
    # round_cast: HW float->int conversion rounds to nearest, so floor(x)
    # is computed as cast(x - 0.5).  CoreSim models trunc; build with
    # round_cast=False to validate in simulation.
    import concourse.mybir as mybir
    from concourse import bacc, tile, library_config

    dt = mybir.dt
    AF = mybir.ActivationFunctionType
    OP = mybir.AluOpType

    nc = bacc.Bacc(
        "TRN2",
        target_bir_lowering=False,
        debug=False,
        enable_asserts=False,
        num_devices=8,
    )

    f32, bf16, i32, i16 = dt.float32, dt.bfloat16, dt.int32, dt.int16

    xin = nc.dram_tensor("xin", [C, NPIMG], bf16, kind="ExternalInput").ap()
    dyt = nc.dram_tensor("dyt", [72, NPIX], f32, kind="ExternalInput").ap()
    dxt = nc.dram_tensor("dxt", [72, NPIX], f32, kind="ExternalInput").ap()
    mkt = nc.dram_tensor("mkt", [72, NPIX], f32, kind="ExternalInput").ap()
    byf = nc.dram_tensor("byf", [72, RB], f32, kind="ExternalInput").ap()
    bxf = nc.dram_tensor("bxf", [72, W], f32, kind="ExternalInput").ap()
    lh0 = nc.dram_tensor("lh0", [128, K * C], bf16, kind="ExternalInput").ap()
    lh1 = nc.dram_tensor("lh1", [128, K * C], bf16, kind="ExternalInput").ap()
    rtp = nc.dram_tensor("rtp", [72, K * 128], bf16, kind="ExternalInput").ap()
    rbt = nc.dram_tensor("rbt", [72, K * 128], bf16, kind="ExternalInput").ap()
    bia = nc.dram_tensor("bia", [C, 1], f32, kind="ExternalInput").ap()
    out = nc.dram_tensor("out", [C, NPIX], f32, kind="ExternalOutput").ap()

    from contextlib import ExitStack

    with tile.TileContext(nc) as tc, ExitStack() as ctx:
        nc.gpsimd.load_library(library_config.ap_gather)

        consts = ctx.enter_context(tc.tile_pool(name="consts", bufs=1))
        lh0_sb = consts.tile([128, K * C], bf16)
        lh1_sb = consts.tile([128, K * C], bf16)
        rtp_sb = consts.tile([72, K * 128], bf16)
        rbt_sb = consts.tile([72, K * 128], bf16)
        bia_sb = consts.tile([C, 1], f32)
        by_sb = consts.tile([72, RB], f32)
        bx_sb = consts.tile([72, W], f32)
        nc.sync.dma_start(lh0_sb, lh0)
        nc.sync.dma_start(lh1_sb, lh1)
        nc.sync.dma_start(rtp_sb, rtp)
        nc.sync.dma_start(rbt_sb, rbt)
        nc.sync.dma_start(bia_sb, bia)
        nc.sync.dma_start(by_sb, byf)
        nc.sync.dma_start(bx_sb, bxf)

        # one-time pair-word packed gather source for the whole core window:
        # partition 16g+8*sh+c' holds two overlapping copies (elem offsets
        # 0/+1, rows shifted by sh) of channel 8g+c' of the padded image
        pimg2_pool = ctx.enter_context(tc.tile_pool(name="pimg2", bufs=1))
        pimg2 = pimg2_pool.tile([128, 2 * NWC2], bf16)
        for g in range(G):
            for sh in range(2):
                p0 = 16 * g + 8 * sh
                off = 144 * sh
                nc.scalar.dma_start(
                    pimg2[p0:p0 + 8, 0:2 * NWC],
                    xin[8 * g:8 * g + 8, off:off + 2 * NWC])
                nc.scalar.dma_start(
                    pimg2[p0:p0 + 8, 2 * NWC:4 * NWC],
                    xin[8 * g:8 * g + 8, off + 1:off + 1 + 2 * NWC])

        prep_pool = ctx.enter_context(tc.tile_pool(name="prep", bufs=1))
        wq_pool = ctx.enter_context(tc.tile_pool(name="wq", bufs=2))
        io_pool = ctx.enter_context(tc.tile_pool(name="io", bufs=2))
        tap_pool = ctx.enter_context(tc.tile_pool(name="tap", bufs=2))
        osb_pool = ctx.enter_context(tc.tile_pool(name="osb", bufs=1))
        psw_pool = ctx.enter_context(
            tc.tile_pool(name="psw", bufs=1, space="PSUM"))
        pso_pool = ctx.enter_context(
            tc.tile_pool(name="pso", bufs=1, space="PSUM"))

        gs_v = pimg2.rearrange("p (w d) -> p w d", d=2)

        for blk in range(NBLK):
            idx16 = io_pool.tile([72, BLKPIX], i16, tag="idx16")
            wqt = wq_pool.tile([72, BLKPIX, 4], bf16, tag="wqt")

            for hf in range(2):
                s0 = HALF * hf
                sl = slice(s0, s0 + HALF)
                gsl = slice(BLKPIX * blk + s0, BLKPIX * blk + s0 + HALF)
                dy_t = io_pool.tile([72, HALF], f32, tag="dy_t", bufs=1)
                dx_t = io_pool.tile([72, HALF], f32, tag="dx_t", bufs=1)
                mk_t = io_pool.tile([72, HALF], f32, tag="mk_t", bufs=1)
                nc.sync.dma_start(dy_t, dyt[:, gsl])
                nc.sync.dma_start(dx_t, dxt[:, gsl])
                nc.sync.dma_start(mk_t, mkt[:, gsl])

                # 7 reused scratch buffers (A..G), liveness-packed
                py = prep_pool.tile([72, HALF], f32, tag="sA", name="py")
                pyc = prep_pool.tile([72, HALF], f32, tag="sB", name="pyc")
                fyi = prep_pool.tile([72, HALF], i32, tag="sA", name="fyi")
                fyf = prep_pool.tile([72, HALF], f32, tag="sC", name="fyf")
                ly = prep_pool.tile([72, HALF], f32, tag="sD", name="ly")
                px = prep_pool.tile([72, HALF], f32, tag="sA", name="px")
                pxc = prep_pool.tile([72, HALF], f32, tag="sB", name="pxc")
                fxi = prep_pool.tile([72, HALF], i32, tag="sA", name="fxi")
                fxf = prep_pool.tile([72, HALF], f32, tag="sE", name="fxf")
                lx = prep_pool.tile([72, HALF], f32, tag="sF", name="lx")
                hx = prep_pool.tile([72, HALF], f32, tag="sA", name="hx")
                hxi = prep_pool.tile([72, HALF], i32, tag="sB", name="hxi")
                hxf = prep_pool.tile([72, HALF], f32, tag="sG", name="hxf")
                epsf = prep_pool.tile([72, HALF], f32, tag="sA", name="epsf")
                t2 = prep_pool.tile([72, HALF], f32, tag="sB", name="t2")
                wordf = prep_pool.tile([72, HALF], f32, tag="sA",
                                       name="wordf")
                smA = prep_pool.tile([72, HALF], f32, tag="sB", name="smA")
                sm = prep_pool.tile([72, HALF], f32, tag="sC", name="sm")
                lxm = prep_pool.tile([72, HALF], f32, tag="sG", name="lxm")
                lyn = prep_pool.tile([72, HALF], f32, tag="sE", name="lyn")

                v = nc.vector
                # py = dy + (yib + ky + 7) + 16*blk   (core-local row coord)
                by_b = by_sb[:, 8 * hf:8 * (hf + 1)].rearrange(
                    "p (y o) -> p y o", o=1).broadcast_to([72, 8, W])
                v.scalar_tensor_tensor(
                    py.rearrange("p (y x) -> p y x", x=W), dy_t.rearrange(
                        "p (y x) -> p y x", x=W), float(RB * blk),
                    by_b, OP.add, OP.add)
                v.tensor_scalar(pyc, py, CLAMP_Y[0], CLAMP_Y[1],
                                OP.max, OP.min)
                shift = -0.5 if round_cast else 0.0
                pys = prep_pool.tile([72, HALF], f32, tag="sF", name="pys")
                v.tensor_scalar_add(pys, pyc, shift)
                v.tensor_copy(fyi, pys)
                v.tensor_copy(fyf, fyi)
                v.tensor_sub(ly, pyc, fyf)

                bx_b = bx_sb.rearrange(
                    "p (o x) -> p o x", o=1).broadcast_to([72, 8, W])
                v.tensor_add(px.rearrange("p (y x) -> p y x", x=W),
                             dx_t.rearrange("p (y x) -> p y x", x=W), bx_b)
                v.tensor_scalar(pxc, px, CLAMP_X[0], CLAMP_X[1],
                                OP.max, OP.min)
                pxs = prep_pool.tile([72, HALF], f32, tag="sG", name="pxs")
                v.tensor_scalar_add(pxs, pxc, shift)
                v.tensor_copy(fxi, pxs)
                v.tensor_copy(fxf, fxi)
                v.tensor_sub(lx, pxc, fxf)

                v.tensor_scalar(hx, fxf, 0.5,
                                -0.25 if round_cast else 0.0,
                                OP.mult, OP.add)
                v.tensor_copy(hxi, hx)
                v.tensor_copy(hxf, hxi)
                # eps = fx - 2*floor(fx/2)
                v.scalar_tensor_tensor(epsf, hxf, -2.0, fxf,
                                       OP.mult, OP.add)
                # word = fy*72 + floor(fx/2) + eps*NWC
                v.scalar_tensor_tensor(t2, epsf, float(NWC), hxf,
                                       OP.mult, OP.add)
                v.scalar_tensor_tensor(wordf, fyf, float(WPAD // 2), t2,
                                       OP.mult, OP.add)
                # idx16 holds each partition's indices in gpsimd-wrapped
                # order: flat s*128+c stores linear pixel c*16+s.  This
                # half's pixels land in column range [64h, 64h+64).
                idx_v = idx16.rearrange("p (s c) -> p c s", s=16)[
                    :, 64 * hf:64 * (hf + 1), :]
                v.tensor_copy(idx_v, wordf)

                v.tensor_scalar_sub(smA, lx, 1.0)     # lx-1
                v.tensor_mul(sm, smA, mk_t)           # (lx-1)m
                v.tensor_mul(lxm, lx, mk_t)           # lx*m
                v.tensor_scalar_sub(lyn, ly, 1.0)     # ly-1
                # signed quad: w00=+q0, w01=-q1, w10=-q2, w11=+q3
                v.tensor_mul(wqt[:, sl, 0], lyn, sm)
                v.tensor_mul(wqt[:, sl, 1], lyn, lxm)
                v.tensor_mul(wqt[:, sl, 2], ly, sm)
                v.tensor_mul(wqt[:, sl, 3], ly, lxm)

            # ---- spread wrapped indices to gpsimd cores: [128, K*128] ----
            idxw = io_pool.tile([128, K * 128], i16, tag="idxw")
            for k in range(K):
                nc.sync.dma_start(
                    idxw[:, 128 * k:128 * (k + 1)],
                    idx16[8 * k:8 * k + 8, :])

            out_ps = pso_pool.tile([C, BLKPIX], mybir.dt.float32, tag="ops")

            for k in range(K):
                wtb = tap_pool.tile([128, BLKPIX, 2], bf16, tag="wtb")
                for hf in range(2):
                    ps = psw_pool.tile([128, BLKPIX], mybir.dt.float32,
                                       tag="psw")
                    for c4 in range(4):
                        pix0 = HALF * hf + 256 * c4
                        psl = ps[:, 512 * c4:512 * (c4 + 1)]
                        nc.tensor.matmul(
                            psl, rtp_sb[:, 128 * k:128 * (k + 1)],
                            wqt[:, pix0:pix0 + 256, 0:2],
                            start=True, stop=False)
                        nc.tensor.matmul(
                            psl, rbt_sb[:, 128 * k:128 * (k + 1)],
                            wqt[:, pix0:pix0 + 256, 2:4],
                            start=False, stop=True)
                    nc.scalar.activation(
                        wtb[:, HALF * hf:HALF * (hf + 1), :], ps, AF.Copy)

                gout = tap_pool.tile([128, BLKPIX, 2], bf16, tag="gout")
                nc.gpsimd.ap_gather(
                    gout, gs_v, idxw[:, 128 * k:128 * (k + 1)],
                    channels=128, num_elems=NWC2, d=2, num_idxs=BLKPIX)

                prod = tap_pool.tile([128, BLKPIX, 2], bf16, tag="prod")
                nc.vector.tensor_mul(prod, gout, wtb)

                for c4 in range(4):
                    psl = out_ps[:, 512 * c4:512 * (c4 + 1)]
                    for q, lh_sb in ((0, lh0_sb), (1, lh1_sb)):
                        nc.tensor.matmul(
                            psl, lh_sb[:, C * k:C * (k + 1)],
                            prod[:, 512 * c4:512 * (c4 + 1), q],
                            start=(k == 0 and q == 0),
                            stop=(k == K - 1 and q == 1))

            out_sb = osb_pool.tile([C, BLKPIX], mybir.dt.float32, tag="osb")
            nc.scalar.activation(out_sb, out_ps, AF.Identity,
                                 bias=bia_sb[:, 0:1])
            nc.sync.dma_start(out[:, BLKPIX * blk:BLKPIX * (blk + 1)],
                              out_sb)

    nc.compile()
    return nc


def _host_constants():
    import ml_dtypes

    # base planes, (k,g) partition order: row index 8k+g
    yib = np.arange(RB, dtype=np.float32)
    xloc = np.arange(W, dtype=np.float32)
    byf = np.zeros((72, RB), np.float32)
    bxf = np.zeros((72, W), np.float32)
    for k in range(K):
        for g in range(G):
            byf[8 * k + g] = yib + (k // 3) + 7.0
            bxf[8 * k + g] = xloc + (k % 3) + 7.0

    rtp = np.zeros((72, K * 128), np.float32)
    rbt = np.zeros((72, K * 128), np.float32)
    for k in range(K):
        for g in range(G):
            rtp[8 * k + g, 128 * k + 16 * g:128 * k + 16 * g + 8] = 1.0
            rbt[8 * k + g, 128 * k + 16 * g + 8:128 * k + 16 * g + 16] = 1.0

    return (byf, bxf, rtp.astype(ml_dtypes.bfloat16),
            rbt.astype(ml_dtypes.bfloat16))


def _host_weights(weight):
    import ml_dtypes
    wr = np.asarray(weight, np.float32).reshape(C, C, K)
    wbf = wr.astype(ml_dtypes.bfloat16).astype(np.float32)
    lh0 = np.zeros((128, K * C), np.float32)
    lh1 = np.zeros((128, K * C), np.float32)
    for k in range(K):
        for g in range(G):
            for cp in range(Cg):
                wv = wbf[:, g * Cg + cp, k]
                lh0[16 * g + cp, C * k:C * (k + 1)] = wv
                lh0[16 * g + 8 + cp, C * k:C * (k + 1)] = -wv
                lh1[16 * g + cp, C * k:C * (k + 1)] = -wv
                lh1[16 * g + 8 + cp, C * k:C * (k + 1)] = wv
    return lh0.astype(ml_dtypes.bfloat16), lh1.astype(ml_dtypes.bfloat16)


def _host_in_maps(input, offset, mask, weight, bias):
    inp = np.ascontiguousarray(np.asarray(input, np.float32))
    off = np.ascontiguousarray(np.asarray(offset, np.float32))
    msk = np.ascontiguousarray(np.asarray(mask, np.float32))
    assert np.abs(off).max() < 6.0, "offset exceeds padding window"

    byf, bxf, rtp, rbt = _host_constants()
    lh0, lh1 = _host_weights(weight)
    bia = np.asarray(bias, np.float32).reshape(C, 1)

    off_r = off.reshape(B, G, K, 2, H, W)
    msk_r = msk.reshape(B, G, K, H, W)

    in_maps = []
    for core in range(8):
        b, h = core // 2, core % 2
        r0 = 64 * h - P
        z = np.zeros((C, PIMG_ROWS, WPAD), np.float32)
        lo, hi = max(0, r0), min(H, r0 + PIMG_ROWS)
        z[:, lo - r0:hi - r0, P:P + W] = inp[b, :, lo:hi, :]
        import ml_dtypes
        xin = np.zeros((C, NPIMG), ml_dtypes.bfloat16)
        xin[:, :PIMG_ROWS * WPAD] = z.reshape(C, -1).astype(ml_dtypes.bfloat16)

        rows = slice(64 * h, 64 * h + 64)
        dyt = np.zeros((72, NPIX), np.float32)
        dxt = np.zeros((72, NPIX), np.float32)
        mkt = np.zeros((72, NPIX), np.float32)
        for k in range(K):
            for g in range(G):
                dyt[8 * k + g] = off_r[b, g, k, 0, rows, :].reshape(-1)
                dxt[8 * k + g] = off_r[b, g, k, 1, rows, :].reshape(-1)
                mkt[8 * k + g] = msk_r[b, g, k, rows, :].reshape(-1)

        in_maps.append({
            "xin": xin, "dyt": dyt, "dxt": dxt, "mkt": mkt,
            "byf": byf, "bxf": bxf,
            "lh0": np.asarray(lh0), "lh1": np.asarray(lh1),
            "rtp": np.asarray(rtp), "rbt": np.asarray(rbt),
            "bia": bia,
        })
    return in_maps


def _get_built(round_cast=True):
    if round_cast not in _BUILT:
        _BUILT[round_cast] = _build_nc(round_cast)
    return _BUILT[round_cast]


def kernel(input, offset, mask, weight, bias):
    from concourse import bass_utils

    nc = _get_built()
    in_maps = _host_in_maps(input, offset, mask, weight, bias)
    res = bass_utils.run_bass_kernel_spmd(nc, in_maps, core_ids=list(range(8)))
    out_full = np.zeros((B, C, H, W), np.float32)
    for core in range(8):
        b, h = core // 2, core % 2
        out_full[b, :, 64 * h:64 * h + 64, :] = (
            res.results[core]["out"].reshape(C, 64, W))
    return out_full

